# revision 29
# baseline (speedup 1.0000x reference)
"""Contextual loss (CX) kernel for Trainium2, 8 NeuronCores.

Sharding: data-parallel over (image, row-half): core c handles image c//2,
pred-rows [ (c%2)*2048, (c%2+1)*2048 ) of the 4096x4096 contextual matrix.

Math (per image, identical to the reference up to per-row constants that
cancel in the row-softmax):
    tc = t - mu,  pc = p - mu              (mu = target's mean feature)
    tc' = tc * invt_h                      (invt_h = Dsqrt(||tc_j||^2) = 1/(2||tc_j||))
    s~_ij  = <pc_i, tc'_j>                 (fp16 matmul; = cos_ij * ||pc_i|| / 2)
    rmax_i = max_j s~_ij                   (fused into the PSUM evacuation)
    smax_i = 4 * rmax_i * invp_h_i         (invp_h per-partition via PE transpose)
    b_i    = 1/(1 - smax_i + EPS)
    e_ij   = exp( (4*b_i*invp_h_i) * s~_ij - b_i*smax_i ),  rs_i = sum_j e_ij
    M_j    = max(M_j, e_ij / rs_i)         (ACT mul + DVE TT-max ping-pong)
Host folds partitions + row-halves, means over j, -log, means over N.
"""

import numpy as np
from contextlib import ExitStack

import concourse.bass as bass
import concourse.bacc as bacc
import concourse.mybir as mybir
import concourse.tile as tile
from concourse.bass_utils import run_bass_kernel_spmd

F32 = mybir.dt.float32
F16 = mybir.dt.float16
AX = mybir.AxisListType.X
ALU = mybir.AluOpType
ACTF = mybir.ActivationFunctionType

N_IMG, C, H, W = 4, 512, 64, 64
HW = H * W              # 4096
R = HW // 2             # 2048 rows per core
KB = C // 128           # 4 contraction blocks
NB = R // 128           # 16 row blocks per core
CH = 512                # matmul free-dim chunk (one PSUM bank)
CC = 2048               # evacuation chunk (four PSUM banks)
NCC = HW // CC          # 2 evac chunks per row block
EPS = 1e-5


def _build_nc():
    nc = bacc.Bacc("TRN2", target_bir_lowering=False, debug=False, num_devices=8)
    t_dram = nc.dram_tensor("t", [C, HW], F32, kind="ExternalInput").ap()
    p_dram = nc.dram_tensor("p", [C, R], F32, kind="ExternalInput").ap()
    m_dram = nc.dram_tensor("m_out", [128, HW], F16, kind="ExternalOutput").ap()

    with tile.TileContext(nc) as tc_ctx, ExitStack() as ctx:
        const = ctx.enter_context(tc_ctx.tile_pool(name="const", bufs=1))

        ones = const.tile([128, 128], F16, tag="ones")
        nc.vector.memset(ones[:], 1.0)

        tct = [const.tile([128, HW], F16, tag=f"tct{k}", name=f"tct{k}") for k in range(KB)]
        pct = [const.tile([128, R], F16, tag=f"pct{k}", name=f"pct{k}") for k in range(KB)]
        invt = const.tile([128, HW], F16, tag="invt")        # 1/(2*colnorm) bcast
        invp = const.tile([128, R], F16, tag="invp")         # 1/(2*rownorm) free layout
        invp_t = const.tile([128, NB], F32, tag="invp_t")    # same, partition layout
        mu = [const.tile([128, 1], F32, tag=f"mu{k}", name=f"mu{k}") for k in range(KB)]

        # ---------------- input DMA (fp32 -> fp16 cast on SWDGE) ----------------
        for k in range(KB):
            nc.gpsimd.dma_start(tct[k][:], t_dram[k * 128:(k + 1) * 128, :])
        for k in range(KB):
            nc.gpsimd.dma_start(pct[k][:], p_dram[k * 128:(k + 1) * 128, :])

        # ---------------- preprocessing ----------------
        sqp = ctx.enter_context(tc_ctx.tile_pool(name="sqp", bufs=1))
        with tc_ctx.tile_pool(name="prepps", bufs=1, space="PSUM") as prepps:
            # PSUM geometry: csa = banks 0-3 (t chunks 0-3), csb = banks 4-7
            # (t chunks 4-7, then reused for pred's colsums).
            cs_a = prepps.tile([128, HW // 2], F32, tag="csa")
            cs_b = prepps.tile([128, HW // 2], F32, tag="csb")

            musum = sqp.tile([128, 1], F32, tag="musum", bufs=2)
            for k in range(KB):
                nc.vector.reduce_sum(musum[:], tct[k][:], axis=AX)
                # store NEGATIVE mean: works as both DVE add-operand and ACT bias
                nc.vector.tensor_scalar(mu[k][:], musum[:], -1.0 / HW, None, ALU.mult)
                # center t in place (fp16, 2x DVE mode)
                nc.vector.tensor_scalar(tct[k][:], tct[k][:], mu[k][:], None, ALU.add)
                sq = sqp.tile([128, HW], F16, tag="sq", bufs=2, name=f"sqt{k}")
                if k < KB - 1:
                    nc.scalar.activation(sq[:], tct[k][:], ACTF.Square)
                else:
                    # last block on DVE: its square gates the whole invt chain
                    nc.vector.tensor_mul(sq[:], tct[k][:], tct[k][:])
                for j in range(4):
                    nc.tensor.matmul(
                        cs_a[:, j * CH:(j + 1) * CH], ones[:],
                        sq[:, j * CH:(j + 1) * CH],
                        start=(k == 0), stop=(k == KB - 1),
                    )
                for j in range(4, 8):
                    nc.tensor.matmul(
                        cs_b[:, (j - 4) * CH:(j - 3) * CH], ones[:],
                        sq[:, j * CH:(j + 1) * CH],
                        start=(k == 0), stop=(k == KB - 1),
                    )

            # invt = rsqrt(colsum) = exp(-0.5*ln(.)) (Rsqrt/Dsqrt unavailable)
            lnt = sqp.tile([128, HW // 2], F32, tag="lnt", bufs=2)
            nc.scalar.activation(lnt[:], cs_a[:], ACTF.Ln)
            nc.scalar.activation(invt[:, :HW // 2], lnt[:], ACTF.Exp, scale=-0.5)
            lnt2 = sqp.tile([128, HW // 2], F32, tag="lnt", bufs=2)
            nc.scalar.activation(lnt2[:], cs_b[:], ACTF.Ln)
            nc.scalar.activation(invt[:, HW // 2:], lnt2[:], ACTF.Exp, scale=-0.5)

            # pred: center with target's mu (ACT, keeps DVE free), squares on
            # GpSimd (idle) except the chain-critical last block on DVE.
            cs_p = prepps.tile([128, R], F32, tag="csb")
            for k in range(KB):
                nc.scalar.activation(
                    pct[k][:], pct[k][:], ACTF.Identity, bias=mu[k][:], scale=1.0
                )
                sqk = sqp.tile([128, R], F16, tag="sqk", bufs=2, name=f"sqp{k}")
                if k < KB - 1:
                    nc.gpsimd.tensor_tensor(sqk[:], pct[k][:], pct[k][:], ALU.mult)
                else:
                    nc.vector.tensor_mul(sqk[:], pct[k][:], pct[k][:])
                for j in range(R // CH):
                    nc.tensor.matmul(
                        cs_p[:, j * CH:(j + 1) * CH], ones[:],
                        sqk[:, j * CH:(j + 1) * CH],
                        start=(k == 0), stop=(k == KB - 1),
                    )
            lnp = sqp.tile([128, R], F32, tag="lnp")
            nc.scalar.activation(lnp[:], cs_p[:], ACTF.Ln)
            nc.scalar.activation(invp[:], lnp[:], ACTF.Exp, scale=-0.5)

        # fold the column scale into t (column-quarter-major so the main
        # loop's first chunks unblock earliest)
        for jh in range(4):
            for k in range(KB):
                nc.vector.tensor_mul(
                    tct[k][:, jh * 1024:(jh + 1) * 1024],
                    tct[k][:, jh * 1024:(jh + 1) * 1024],
                    invt[:, jh * 1024:(jh + 1) * 1024],
                )

        # ---------------- main loop ----------------
        main = ctx.enter_context(tc_ctx.tile_pool(name="main", bufs=2))
        stats = ctx.enter_context(tc_ctx.tile_pool(name="stats", bufs=2))
        mainps = ctx.enter_context(tc_ctx.tile_pool(name="mainps", bufs=2, space="PSUM"))

        # invp free-layout -> partition layout: DMA-transpose each 128-wide
        # slice (all partitions equal, so column 0 of the transpose is the
        # per-partition vector). Runs on the idle DMA queues; the tiny column
        # copies are interleaved into the loop two blocks ahead of use.
        tp_tiles = []
        for ib in range(NB):
            tpt = sqp.tile([128, 128], F16, tag=f"tp{ib}", name=f"tp{ib}")
            nc.sync.dma_start_transpose(tpt[:], invp[:, ib * 128:(ib + 1) * 128])
            tp_tiles.append(tpt)

        def copy_invp(ib):
            # store NEGATED invp so the stats chain below saves two ops
            nc.vector.tensor_scalar(
                invp_t[:, ib:ib + 1], tp_tiles[ib][:, 0:1], -1.0, None, ALU.mult
            )

        copy_invp(0)
        copy_invp(1)

        m_prev = main.tile([128, HW], F16, tag="m")
        nc.vector.memset(m_prev[:], 0.0)

        e_tiles = [None] * NB
        rs_tiles = [None] * NB
        rinv_tiles = [None] * NB

        def finalize(ib):
            """rinv = 1/rs (its exp finished an iteration ago, so the DVE
            FIFO never blocks), e' = e*rinv on ACT, column-max fold on DVE."""
            nonlocal m_prev
            nc.vector.reciprocal(rinv_tiles[ib][:], rs_tiles[ib][:])
            ep = main.tile([128, HW], F16, tag="ep")
            nc.scalar.mul(ep[:], e_tiles[ib][:], rinv_tiles[ib][:])
            m_cur = main.tile([128, HW], F16, tag="m")
            nc.vector.tensor_tensor(m_cur[:], ep[:], m_prev[:], ALU.max)
            m_prev = m_cur

        for ib in range(NB):
            s_t = main.tile([128, HW], F16, tag="s")
            e_t = main.tile([128, HW], F16, tag="e", bufs=3)
            cmax = stats.tile([128, NCC], F32, tag="cmax")

            for c in range(NCC):
                ps = mainps.tile([128, CC], F32, tag="ps")
                for half in range(CC // CH):
                    for kc in range(KB):
                        nc.tensor.matmul(
                            ps[:, half * CH:(half + 1) * CH],
                            pct[kc][:, ib * 128:(ib + 1) * 128],
                            tct[kc][:, (c * (CC // CH) + half) * CH:
                                      (c * (CC // CH) + half + 1) * CH],
                            start=(kc == 0), stop=(kc == KB - 1),
                        )
                # fused PSUM->SBUF copy + row-max accumulation on DVE
                nc.vector.tensor_scalar(
                    s_t[:, c * CC:(c + 1) * CC], ps[:], 1.0, None, ALU.mult, ALU.max,
                    accum_out=cmax[:, c:c + 1],
                )
            if ib + 2 < NB:
                copy_invp(ib + 2)

            rawmax = stats.tile([128, 1], F32, tag="rawmax")
            tmp = stats.tile([128, 1], F32, tag="tmp")
            b_t = stats.tile([128, 1], F32, tag="b")
            scale_e = stats.tile([128, 1], F32, tag="scale_e")
            bias_e = stats.tile([128, 1], F32, tag="bias_e")
            rs = stats.tile([128, 1], F32, tag="rs", bufs=3)
            rinv = stats.tile([128, 1], F32, tag="rinv", bufs=3)

            nc.vector.reduce_max(rawmax[:], cmax[:], axis=AX)
            # tmp = 1 + EPS - smax,  smax = rawmax*invp  (invp_t is negated)
            nc.vector.tensor_scalar(
                tmp[:], rawmax[:], invp_t[:, ib:ib + 1], 1.0 + EPS, ALU.mult, ALU.add
            )
            nc.vector.reciprocal(b_t[:], tmp[:])
            # scale_e = b*invp = -b*invp_t
            nc.vector.scalar_tensor_tensor(
                scale_e[:], b_t[:], -1.0, invp_t[:, ib:ib + 1], ALU.mult, ALU.mult
            )
            # bias_e = -b*smax = (tmp - (1+EPS)) * b
            nc.vector.scalar_tensor_tensor(
                bias_e[:], tmp[:], -(1.0 + EPS), b_t[:], ALU.add, ALU.mult
            )
            # two-iteration-delayed normalization: by the time the DVE FIFO
            # reaches the reciprocal, its exp (and accumulator read) finished
            # a full iteration ago, so nothing bubbles
            if ib > 1:
                finalize(ib - 2)
            nc.scalar.activation(
                e_t[:], s_t[:], ACTF.Exp, bias=bias_e[:], scale=scale_e[:],
                accum_out=rs[:],
            )
            e_tiles[ib] = e_t
            rs_tiles[ib] = rs
            rinv_tiles[ib] = rinv

        # last blocks: normalize + fold + store in column halves so the
        # output DMA overlaps the remaining compute
        finalize(NB - 2)
        HH = HW // 2
        nc.vector.reciprocal(rinv_tiles[NB - 1][:], rs_tiles[NB - 1][:])
        ep = main.tile([128, HW], F16, tag="ep")
        m_cur = main.tile([128, HW], F16, tag="m")
        for h in range(2):
            sl = slice(h * HH, (h + 1) * HH)
            nc.scalar.mul(ep[:, sl], e_tiles[NB - 1][:, sl], rinv_tiles[NB - 1][:])
            nc.vector.tensor_tensor(m_cur[:, sl], ep[:, sl], m_prev[:, sl], ALU.max)
            nc.sync.dma_start(m_dram[:, sl], m_cur[:, sl])
    nc.compile()
    return nc


_NC_CACHE = {}


def _get_nc():
    if "nc" not in _NC_CACHE:
        _NC_CACHE["nc"] = _build_nc()
    return _NC_CACHE["nc"]


def kernel(pred, target, _trace=False):
    pred = np.asarray(pred, dtype=np.float32).reshape(N_IMG, C, HW)
    target = np.asarray(target, dtype=np.float32).reshape(N_IMG, C, HW)
    nc = _get_nc()
    in_maps = []
    for core in range(8):
        img, half = divmod(core, 2)
        in_maps.append({
            "t": np.ascontiguousarray(target[img]),
            "p": np.ascontiguousarray(pred[img, :, half * R:(half + 1) * R]),
        })
    res = run_bass_kernel_spmd(nc, in_maps, list(range(8)), trace=_trace)
    losses = []
    for img in range(N_IMG):
        m0 = res.results[2 * img]["m_out"].astype(np.float32).max(axis=0)
        m1 = res.results[2 * img + 1]["m_out"].astype(np.float32).max(axis=0)
        cx = np.maximum(m0, m1).mean()
        losses.append(-np.log(cx + EPS))
    out = np.float32(np.mean(losses))
    if _trace:
        return out, res
    return out


# revision 30
# speedup vs baseline: 1.1740x; 1.1740x over previous
"""Contextual loss (CX) kernel for Trainium2, 8 NeuronCores.

Sharding: data-parallel over (image, row-half): core c handles image c//2,
pred-rows [ (c%2)*2048, (c%2+1)*2048 ) of the 4096x4096 contextual matrix.

Math (per image, identical to the reference up to per-row constants that
cancel in the row-softmax):
    tc = t - mu,  pc = p - mu              (mu = target's mean feature)
    tc' = tc * invt_h                      (invt_h = Dsqrt(||tc_j||^2) = 1/(2||tc_j||))
    s~_ij  = <pc_i, tc'_j>                 (fp16 matmul; = cos_ij * ||pc_i|| / 2)
    rmax_i = max_j s~_ij                   (fused into the PSUM evacuation)
    smax_i = 4 * rmax_i * invp_h_i         (invp_h per-partition via PE transpose)
    b_i    = 1/(1 - smax_i + EPS)
    e_ij   = exp( (4*b_i*invp_h_i) * s~_ij - b_i*smax_i ),  rs_i = sum_j e_ij
    M_j    = max(M_j, e_ij / rs_i)         (ACT mul + DVE TT-max ping-pong)
Host folds partitions + row-halves, means over j, -log, means over N.
"""

import numpy as np
from contextlib import ExitStack

import concourse.bass as bass
import concourse.bacc as bacc
import concourse.mybir as mybir
import concourse.tile as tile
from concourse.bass_utils import run_bass_kernel_spmd

F32 = mybir.dt.float32
F16 = mybir.dt.float16
AX = mybir.AxisListType.X
ALU = mybir.AluOpType
ACTF = mybir.ActivationFunctionType

N_IMG, C, H, W = 4, 512, 64, 64
HW = H * W              # 4096
R = HW // 2             # 2048 rows per core
KB = C // 128           # 4 contraction blocks
NB = R // 128           # 16 row blocks per core
CH = 512                # matmul free-dim chunk (one PSUM bank)
CC = 1024               # evacuation chunk (two PSUM banks)
NCC = HW // CC          # 4 evac chunks per row block
EPS = 1e-5


def _build_nc():
    nc = bacc.Bacc("TRN2", target_bir_lowering=False, debug=False, num_devices=8)
    t_dram = nc.dram_tensor("t", [C, HW], F32, kind="ExternalInput").ap()
    p_dram = nc.dram_tensor("p", [C, R], F32, kind="ExternalInput").ap()
    m_dram = nc.dram_tensor("m_out", [128, HW], F16, kind="ExternalOutput").ap()

    with tile.TileContext(nc) as tc_ctx, ExitStack() as ctx:
        const = ctx.enter_context(tc_ctx.tile_pool(name="const", bufs=1))

        ones = const.tile([128, 128], F16, tag="ones")
        nc.vector.memset(ones[:], 1.0)

        tct = [const.tile([128, HW], F16, tag=f"tct{k}", name=f"tct{k}") for k in range(KB)]
        pct = [const.tile([128, R], F16, tag=f"pct{k}", name=f"pct{k}") for k in range(KB)]
        invt = const.tile([128, HW], F16, tag="invt")        # 1/(2*colnorm) bcast
        invp = const.tile([128, R], F16, tag="invp")         # 1/(2*rownorm) free layout
        invp_t = const.tile([128, NB], F32, tag="invp_t")    # same, partition layout
        mu = [const.tile([128, 1], F32, tag=f"mu{k}", name=f"mu{k}") for k in range(KB)]

        # ---------------- input DMA (fp32 -> fp16 cast on SWDGE) ----------------
        for k in range(KB):
            nc.gpsimd.dma_start(tct[k][:], t_dram[k * 128:(k + 1) * 128, :])
        for k in range(KB):
            nc.gpsimd.dma_start(pct[k][:], p_dram[k * 128:(k + 1) * 128, :])

        # ---------------- preprocessing ----------------
        sqp = ctx.enter_context(tc_ctx.tile_pool(name="sqp", bufs=1))
        with tc_ctx.tile_pool(name="prepps", bufs=1, space="PSUM") as prepps:
            # PSUM geometry: csa = banks 0-3 (t chunks 0-3), csb = banks 4-7
            # (t chunks 4-7, then reused for pred's colsums).
            cs_a = prepps.tile([128, HW // 2], F32, tag="csa")
            cs_b = prepps.tile([128, HW // 2], F32, tag="csb")

            musum = sqp.tile([128, 1], F32, tag="musum", bufs=2)
            for k in range(KB):
                nc.vector.reduce_sum(musum[:], tct[k][:], axis=AX)
                # store NEGATIVE mean: works as both DVE add-operand and ACT bias
                nc.vector.tensor_scalar(mu[k][:], musum[:], -1.0 / HW, None, ALU.mult)
                # center t in place (fp16, 2x DVE mode)
                nc.vector.tensor_scalar(tct[k][:], tct[k][:], mu[k][:], None, ALU.add)
                sq = sqp.tile([128, HW], F16, tag="sq", bufs=2, name=f"sqt{k}")
                if k < KB - 1:
                    nc.scalar.activation(sq[:], tct[k][:], ACTF.Square)
                else:
                    # last block on DVE: its square gates the whole invt chain
                    nc.vector.tensor_mul(sq[:], tct[k][:], tct[k][:])
                for j in range(4):
                    nc.tensor.matmul(
                        cs_a[:, j * CH:(j + 1) * CH], ones[:],
                        sq[:, j * CH:(j + 1) * CH],
                        start=(k == 0), stop=(k == KB - 1),
                    )
                for j in range(4, 8):
                    nc.tensor.matmul(
                        cs_b[:, (j - 4) * CH:(j - 3) * CH], ones[:],
                        sq[:, j * CH:(j + 1) * CH],
                        start=(k == 0), stop=(k == KB - 1),
                    )

            # invt = rsqrt(colsum) = exp(-0.5*ln(.)) (Rsqrt/Dsqrt unavailable)
            lnt = sqp.tile([128, HW // 2], F32, tag="lnt", bufs=2)
            nc.scalar.activation(lnt[:], cs_a[:], ACTF.Ln)
            nc.scalar.activation(invt[:, :HW // 2], lnt[:], ACTF.Exp, scale=-0.5)
            lnt2 = sqp.tile([128, HW // 2], F32, tag="lnt", bufs=2)
            nc.scalar.activation(lnt2[:], cs_b[:], ACTF.Ln)
            nc.scalar.activation(invt[:, HW // 2:], lnt2[:], ACTF.Exp, scale=-0.5)

            # pred: center with target's mu (ACT, keeps DVE free), squares on
            # GpSimd (idle) except the chain-critical last block on DVE.
            cs_p = prepps.tile([128, R], F32, tag="csb")
            for k in range(KB):
                nc.scalar.activation(
                    pct[k][:], pct[k][:], ACTF.Identity, bias=mu[k][:], scale=1.0
                )
                sqk = sqp.tile([128, R], F16, tag="sqk", bufs=2, name=f"sqp{k}")
                if k < KB - 1:
                    nc.gpsimd.tensor_tensor(sqk[:], pct[k][:], pct[k][:], ALU.mult)
                else:
                    nc.vector.tensor_mul(sqk[:], pct[k][:], pct[k][:])
                for j in range(R // CH):
                    nc.tensor.matmul(
                        cs_p[:, j * CH:(j + 1) * CH], ones[:],
                        sqk[:, j * CH:(j + 1) * CH],
                        start=(k == 0), stop=(k == KB - 1),
                    )
            lnp = sqp.tile([128, R], F32, tag="lnp")
            nc.scalar.activation(lnp[:], cs_p[:], ACTF.Ln)
            nc.scalar.activation(invp[:], lnp[:], ACTF.Exp, scale=-0.5)

        # fold the column scale into t (column-quarter-major so the main
        # loop's first chunks unblock earliest)
        for jh in range(4):
            for k in range(KB):
                nc.vector.tensor_mul(
                    tct[k][:, jh * 1024:(jh + 1) * 1024],
                    tct[k][:, jh * 1024:(jh + 1) * 1024],
                    invt[:, jh * 1024:(jh + 1) * 1024],
                )

        # ---------------- main loop ----------------
        main = ctx.enter_context(tc_ctx.tile_pool(name="main", bufs=2))
        stats = ctx.enter_context(tc_ctx.tile_pool(name="stats", bufs=2))
        mainps = ctx.enter_context(tc_ctx.tile_pool(name="mainps", bufs=4, space="PSUM"))

        # invp free-layout -> partition layout: DMA-transpose each 128-wide
        # slice (all partitions equal, so column 0 of the transpose is the
        # per-partition vector). Runs on the idle DMA queues; the tiny column
        # copies are interleaved into the loop two blocks ahead of use.
        tp_tiles = []
        for ib in range(NB):
            tpt = sqp.tile([128, 128], F16, tag=f"tp{ib}", name=f"tp{ib}")
            nc.sync.dma_start_transpose(tpt[:], invp[:, ib * 128:(ib + 1) * 128])
            tp_tiles.append(tpt)

        def copy_invp(ib):
            # store NEGATED invp so the stats chain below saves two ops
            nc.vector.tensor_scalar(
                invp_t[:, ib:ib + 1], tp_tiles[ib][:, 0:1], -1.0, None, ALU.mult
            )

        copy_invp(0)
        copy_invp(1)

        m_prev = main.tile([128, HW], F16, tag="m")
        nc.vector.memset(m_prev[:], 0.0)

        e_tiles = [None] * NB
        rs_tiles = [None] * NB
        rinv_tiles = [None] * NB

        def finalize(ib):
            """e' = e*rinv on ACT, then column-max fold on DVE (ping-pong)."""
            nonlocal m_prev
            ep = main.tile([128, HW], F16, tag="ep")
            nc.scalar.mul(ep[:], e_tiles[ib][:], rinv_tiles[ib][:])
            m_cur = main.tile([128, HW], F16, tag="m")
            nc.vector.tensor_tensor(m_cur[:], ep[:], m_prev[:], ALU.max)
            m_prev = m_cur

        for ib in range(NB):
            s_t = main.tile([128, HW], F16, tag="s")
            e_t = main.tile([128, HW], F16, tag="e")
            cmax = stats.tile([128, NCC], F32, tag="cmax")

            for c in range(NCC):
                ps = mainps.tile([128, CC], F32, tag="ps")
                for half in range(CC // CH):
                    for kc in range(KB):
                        nc.tensor.matmul(
                            ps[:, half * CH:(half + 1) * CH],
                            pct[kc][:, ib * 128:(ib + 1) * 128],
                            tct[kc][:, (c * (CC // CH) + half) * CH:
                                      (c * (CC // CH) + half + 1) * CH],
                            start=(kc == 0), stop=(kc == KB - 1),
                        )
                # fused PSUM->SBUF copy + row-max accumulation on DVE
                nc.vector.tensor_scalar(
                    s_t[:, c * CC:(c + 1) * CC], ps[:], 1.0, None, ALU.mult, ALU.max,
                    accum_out=cmax[:, c:c + 1],
                )
            if ib + 2 < NB:
                copy_invp(ib + 2)

            rawmax = stats.tile([128, 1], F32, tag="rawmax")
            tmp = stats.tile([128, 1], F32, tag="tmp")
            b_t = stats.tile([128, 1], F32, tag="b")
            scale_e = stats.tile([128, 1], F32, tag="scale_e")
            bias_e = stats.tile([128, 1], F32, tag="bias_e")
            rs = stats.tile([128, 1], F32, tag="rs")
            rinv = stats.tile([128, 1], F32, tag="rinv")

            nc.vector.reduce_max(rawmax[:], cmax[:], axis=AX)
            # tmp = 1 + EPS - smax,  smax = rawmax*invp  (invp_t is negated)
            nc.vector.tensor_scalar(
                tmp[:], rawmax[:], invp_t[:, ib:ib + 1], 1.0 + EPS, ALU.mult, ALU.add
            )
            nc.vector.reciprocal(b_t[:], tmp[:])
            # scale_e = b*invp = -b*invp_t
            nc.vector.scalar_tensor_tensor(
                scale_e[:], b_t[:], -1.0, invp_t[:, ib:ib + 1], ALU.mult, ALU.mult
            )
            # bias_e = -b*smax = (tmp - (1+EPS)) * b
            nc.vector.scalar_tensor_tensor(
                bias_e[:], tmp[:], -(1.0 + EPS), b_t[:], ALU.add, ALU.mult
            )
            nc.scalar.activation(
                e_t[:], s_t[:], ACTF.Exp, bias=bias_e[:], scale=scale_e[:],
                accum_out=rs[:],
            )
            nc.vector.reciprocal(rinv[:], rs[:])
            e_tiles[ib] = e_t
            rs_tiles[ib] = rs
            rinv_tiles[ib] = rinv
            # one-iteration-delayed normalization keeps the ACT FIFO flowing
            if ib > 0:
                finalize(ib - 1)

        # last blocks: normalize + fold + store in column halves so the
        # output DMA overlaps the remaining compute
        HH = HW // 2
        ep = main.tile([128, HW], F16, tag="ep")
        m_cur = main.tile([128, HW], F16, tag="m")
        for h in range(2):
            sl = slice(h * HH, (h + 1) * HH)
            nc.scalar.mul(ep[:, sl], e_tiles[NB - 1][:, sl], rinv_tiles[NB - 1][:])
            nc.vector.tensor_tensor(m_cur[:, sl], ep[:, sl], m_prev[:, sl], ALU.max)
            nc.sync.dma_start(m_dram[:, sl], m_cur[:, sl])
    nc.compile()
    return nc


_NC_CACHE = {}


def _get_nc():
    if "nc" not in _NC_CACHE:
        _NC_CACHE["nc"] = _build_nc()
    return _NC_CACHE["nc"]


def kernel(pred, target, _trace=False):
    pred = np.asarray(pred, dtype=np.float32).reshape(N_IMG, C, HW)
    target = np.asarray(target, dtype=np.float32).reshape(N_IMG, C, HW)
    nc = _get_nc()
    in_maps = []
    for core in range(8):
        img, half = divmod(core, 2)
        in_maps.append({
            "t": np.ascontiguousarray(target[img]),
            "p": np.ascontiguousarray(pred[img, :, half * R:(half + 1) * R]),
        })
    res = run_bass_kernel_spmd(nc, in_maps, list(range(8)), trace=_trace)
    losses = []
    for img in range(N_IMG):
        m0 = res.results[2 * img]["m_out"].astype(np.float32).max(axis=0)
        m1 = res.results[2 * img + 1]["m_out"].astype(np.float32).max(axis=0)
        cx = np.maximum(m0, m1).mean()
        losses.append(-np.log(cx + EPS))
    out = np.float32(np.mean(losses))
    if _trace:
        return out, res
    return out
